# revision 6
# baseline (speedup 1.0000x reference)
"""Embedding lookup (nn_LookupNetwork) on 8 Trainium2 NeuronCores.

Strategy: vocab-sharded gather. The 100000x128 f32 table is row-sharded
across the 8 cores (12500 rows each), so per-core row indices fit the
int16 index format of the batched SWDGE gather instruction
(InstDMAGatherAnt), which gathers one 512 B table row per index with a
single descriptor instead of one indirect DMA instruction per SBUF
column. The host routes each of the 819200 lookups to the core owning
its table row (sentinel -1 lookups are routed nowhere and stay zero),
ships per-core local indices in the gather's wrapped-by-16 layout, and
scatters the per-core gathered rows back to their original positions.
On-device each core is a pure copy engine: 92 tiles x 1024 indices,
dma_gather HBM->SBUF then an HWDGE store SBUF->HBM, triple buffered.
"""

import sys

sys.path.insert(0, "/opt/trn_rl_repo")

from contextlib import ExitStack

import numpy as np

import concourse.bacc as bacc
import concourse.bass as bass
import concourse.mybir as mybir
import concourse.tile as tile
from concourse.bass_utils import run_bass_kernel_spmd

VOCAB, D = 100000, 128
BATCH, HIST = 4096, 200
NCORES = 8
P = 128
SHARD = VOCAB // NCORES  # table rows per core
TN = 1024  # lookups per gather tile (HW ucode rejects >1024 idx per gather)
NI = 92 * TN  # padded lookup capacity per core (max shard load ~92.5k)

_nc_cache = {}


def build_nc(bufs=3, reps=1):
    """reps > 1 repeats the whole body on-device (for (t_R - t_1)/(R-1)
    exec timing — the bass2jax hook only allows one bass_exec per jit)."""
    nc = bacc.Bacc(
        "TRN2", target_bir_lowering=False, debug=False, enable_asserts=False
    )
    idx_d = nc.dram_tensor(
        "idx", [P, NI // 16], mybir.dt.int16, kind="ExternalInput"
    ).ap()
    tab_d = nc.dram_tensor(
        "tab", [SHARD, D], mybir.dt.float32, kind="ExternalInput"
    ).ap()
    out_d = nc.dram_tensor(
        "out", [NI, D], mybir.dt.float32, kind="ExternalOutput"
    ).ap()

    with tile.TileContext(nc) as tc:
        with ExitStack() as ctx:
            ipool = ctx.enter_context(tc.tile_pool(name="ipool", bufs=2))
            gpool = ctx.enter_context(tc.tile_pool(name="gpool", bufs=bufs))

            for _ in range(reps):
                idx_t = ipool.tile([P, NI // 16], mybir.dt.int16)
                nc.sync.dma_start(idx_t[:], idx_d)

                for t in range(NI // TN):
                    g = gpool.tile([P, (TN // P) * D], mybir.dt.float32)
                    g3 = g[:].rearrange("p (c d) -> p c d", d=D)
                    # Gathered slot i lands at [i % 128, i // 128, :].
                    nc.gpsimd.dma_gather(
                        out_ap=g3,
                        in_ap=tab_d,
                        idxs_ap=idx_t[:, t * (TN // 16) : (t + 1) * (TN // 16)],
                        num_idxs=TN,
                        num_idxs_reg=TN,
                        elem_size=D,
                    )
                    dst = out_d[t * TN : (t + 1) * TN, :].rearrange(
                        "(c p) d -> p c d", p=P
                    )
                    nc.sync.dma_start(dst, g3)
    nc.compile()
    return nc


def _get_nc(reps=1):
    if reps not in _nc_cache:
        _nc_cache[reps] = build_nc(reps=reps)
    return _nc_cache[reps]


def _prep(input_batch, table):
    """Route lookups to vocab-shard owners; build per-core device inputs."""
    idx = np.asarray(input_batch).reshape(-1).astype(np.int64)
    tab = np.ascontiguousarray(np.asarray(table, dtype=np.float32))
    in_maps, places = [], []
    for c in range(NCORES):
        lo = c * SHARD
        pos = np.nonzero((idx >= lo) & (idx < lo + SHARD))[0]
        over = None
        if len(pos) > NI:  # capacity overflow: excess handled on host
            over = pos[NI:]
            pos = pos[:NI]
        local = (idx[pos] - lo).astype(np.int16)
        buf = np.zeros(NI, np.int16)
        buf[: len(local)] = local
        wrapped = np.ascontiguousarray(
            np.tile(buf.reshape(NI // 16, 16).T, (NCORES, 1))
        )
        in_maps.append({"idx": wrapped, "tab": tab[lo : lo + SHARD]})
        places.append((pos, over))
    return in_maps, places, idx, tab


def kernel(input_batch, table):
    nc = _get_nc()
    in_maps, places, idx, tab = _prep(input_batch, table)
    res = run_bass_kernel_spmd(nc, in_maps, list(range(NCORES)))
    out = np.zeros((BATCH * HIST, D), np.float32)
    for c in range(NCORES):
        pos, over = places[c]
        out[pos] = res.results[c]["out"][: len(pos)]
        if over is not None:
            out[over] = tab[idx[over]]
    return out.reshape(BATCH, HIST, D)


def bench(input_batch, table, reps=20, nc=None, chain=1):
    """Time repeated on-device executions (inputs device-resident, no
    donation, no host transfers in the timed region). `chain` repeats the
    kernel body on-device inside one bass program; time two chain values
    and divide the difference to cancel dispatch overhead. Returns wall
    seconds (min over reps) including the axon dispatch round trip."""
    import time

    import jax
    from jax.sharding import Mesh, NamedSharding, PartitionSpec
    from jax.experimental.shard_map import shard_map

    from concourse import bass2jax
    from concourse.bass2jax import (
        _bass_exec_p,
        install_neuronx_cc_hook,
        partition_id_tensor,
    )

    if nc is None:
        nc = _get_nc(reps=chain)
    install_neuronx_cc_hook()
    in_maps, _, _, _ = _prep(input_batch, table)

    partition_name = (
        nc.partition_id_tensor.name if nc.partition_id_tensor else None
    )
    in_names, out_names, out_avals, zero_outs = [], [], [], []
    for alloc in nc.m.functions[0].allocations:
        if not isinstance(alloc, mybir.MemoryLocationSet):
            continue
        name = alloc.memorylocations[0].name
        if alloc.kind == "ExternalInput":
            if name != partition_name:
                in_names.append(name)
        elif alloc.kind == "ExternalOutput":
            out_names.append(name)
            shape = tuple(alloc.tensor_shape)
            dtype = mybir.dt.np(alloc.dtype)
            out_avals.append(jax.core.ShapedArray(shape, dtype))
            zero_outs.append(np.zeros(shape, dtype))
    n_params = len(in_names)
    all_in_names = in_names + out_names
    if partition_name is not None:
        all_in_names = all_in_names + [partition_name]

    def _body(*args):
        ins_only = list(args[:n_params])
        outs = list(args[n_params:])
        pid = [partition_id_tensor()] if partition_name is not None else []
        operands = ins_only + outs + pid
        outs = list(
            _bass_exec_p.bind(
                *operands,
                out_avals=tuple(out_avals),
                in_names=tuple(all_in_names),
                out_names=tuple(out_names),
                lowering_input_output_aliases=(),
                sim_require_finite=True,
                sim_require_nnan=True,
                nc=nc,
            )
        )
        return tuple(outs)

    devices = jax.devices()[:NCORES]
    mesh = Mesh(np.asarray(devices), ("core",))
    nshard = NamedSharding(mesh, PartitionSpec("core"))
    sharded = jax.jit(
        shard_map(
            _body,
            mesh=mesh,
            in_specs=(PartitionSpec("core"),) * (n_params + len(out_names)),
            out_specs=(PartitionSpec("core"),) * len(out_names),
            check_rep=False,
        ),
        keep_unused=True,
    )
    concat_in = [
        np.concatenate([np.asarray(in_maps[c][nm]) for c in range(NCORES)], axis=0)
        for nm in in_names
    ]
    concat_zeros = [
        np.zeros((NCORES * z.shape[0], *z.shape[1:]), z.dtype) for z in zero_outs
    ]
    dev_args = [jax.device_put(a, nshard) for a in concat_in + concat_zeros]
    jax.block_until_ready(dev_args)
    # warmup (compiles NEFF on first call)
    out = sharded(*dev_args)
    jax.block_until_ready(out)
    times = []
    for _ in range(reps):
        t0 = time.perf_counter()
        out = sharded(*dev_args)
        jax.block_until_ready(out)
        times.append(time.perf_counter() - t0)
    return min(times), times, out


# revision 7
# speedup vs baseline: 4.5670x; 4.5670x over previous
"""Embedding lookup (nn_LookupNetwork) on 8 Trainium2 NeuronCores.

Strategy: vocab-sharded gather. The 100000x128 f32 table is row-sharded
across the 8 cores (12500 rows each), so per-core row indices fit the
int16 index format of the batched SWDGE gather instruction
(InstDMAGatherAnt), which gathers one 512 B table row per index with a
single descriptor instead of one indirect DMA instruction per SBUF
column. The host routes each of the 819200 lookups to the core owning
its table row (sentinel -1 lookups are routed nowhere and stay zero),
ships per-core local indices in the gather's wrapped-by-16 layout, and
scatters the per-core gathered rows back to their original positions.
On-device each core is a pure copy engine: 92 tiles x 1024 indices,
dma_gather HBM->SBUF then an HWDGE store SBUF->HBM, triple buffered.
"""

import sys

sys.path.insert(0, "/opt/trn_rl_repo")

from contextlib import ExitStack

import numpy as np

import concourse.bacc as bacc
import concourse.bass as bass
import concourse.mybir as mybir
import concourse.tile as tile
from concourse.bass_utils import run_bass_kernel_spmd

VOCAB, D = 100000, 128
BATCH, HIST = 4096, 200
NCORES = 8
P = 128
SHARD = VOCAB // NCORES  # table rows per core
TN = 1024  # lookups per gather tile (HW ucode rejects >1024 idx per gather)
NI = 92 * TN  # padded lookup capacity per core (max shard load ~92.5k)

_nc_cache = {}


def build_nc(bufs=6, reps=1):
    """reps > 1 repeats the whole body on-device (for (t_R - t_1)/(R-1)
    exec timing — the bass2jax hook only allows one bass_exec per jit)."""
    nc = bacc.Bacc(
        "TRN2",
        target_bir_lowering=False,
        debug=False,
        enable_asserts=False,
        num_swdge_queues=4,
    )
    idx_d = nc.dram_tensor(
        "idx", [P, NI // 16], mybir.dt.int16, kind="ExternalInput"
    ).ap()
    tab_d = nc.dram_tensor(
        "tab", [SHARD, D], mybir.dt.float32, kind="ExternalInput"
    ).ap()
    out_d = nc.dram_tensor(
        "out", [NI, D], mybir.dt.float32, kind="ExternalOutput"
    ).ap()

    with tile.TileContext(nc) as tc:
        with ExitStack() as ctx:
            ipool = ctx.enter_context(tc.tile_pool(name="ipool", bufs=2))
            gpool = ctx.enter_context(tc.tile_pool(name="gpool", bufs=bufs))

            for _ in range(reps):
                idx_t = ipool.tile([P, NI // 16], mybir.dt.int16)
                nc.sync.dma_start(idx_t[:], idx_d)

                for t in range(NI // TN):
                    g = gpool.tile([P, (TN // P) * D], mybir.dt.float32)
                    g3 = g[:].rearrange("p (c d) -> p c d", d=D)
                    # Gathered slot i lands at [i % 128, i // 128, :].
                    nc.gpsimd.dma_gather(
                        out_ap=g3,
                        in_ap=tab_d,
                        idxs_ap=idx_t[:, t * (TN // 16) : (t + 1) * (TN // 16)],
                        num_idxs=TN,
                        num_idxs_reg=TN,
                        elem_size=D,
                        queue_num=t % 4,
                    )
                    dst = out_d[t * TN : (t + 1) * TN, :].rearrange(
                        "(c p) d -> p c d", p=P
                    )
                    nc.sync.dma_start(dst, g3)
    nc.compile()
    return nc


def _get_nc(reps=1):
    if reps not in _nc_cache:
        _nc_cache[reps] = build_nc(reps=reps)
    return _nc_cache[reps]


def _prep(input_batch, table):
    """Route lookups to vocab-shard owners; build per-core device inputs."""
    idx = np.asarray(input_batch).reshape(-1).astype(np.int64)
    tab = np.ascontiguousarray(np.asarray(table, dtype=np.float32))
    in_maps, places = [], []
    for c in range(NCORES):
        lo = c * SHARD
        pos = np.nonzero((idx >= lo) & (idx < lo + SHARD))[0]
        over = None
        if len(pos) > NI:  # capacity overflow: excess handled on host
            over = pos[NI:]
            pos = pos[:NI]
        local = (idx[pos] - lo).astype(np.int16)
        buf = np.zeros(NI, np.int16)
        buf[: len(local)] = local
        wrapped = np.ascontiguousarray(
            np.tile(buf.reshape(NI // 16, 16).T, (NCORES, 1))
        )
        in_maps.append({"idx": wrapped, "tab": tab[lo : lo + SHARD]})
        places.append((pos, over))
    return in_maps, places, idx, tab


def kernel(input_batch, table):
    nc = _get_nc()
    in_maps, places, idx, tab = _prep(input_batch, table)
    res = run_bass_kernel_spmd(nc, in_maps, list(range(NCORES)))
    out = np.zeros((BATCH * HIST, D), np.float32)
    for c in range(NCORES):
        pos, over = places[c]
        out[pos] = res.results[c]["out"][: len(pos)]
        if over is not None:
            out[over] = tab[idx[over]]
    return out.reshape(BATCH, HIST, D)


def bench(input_batch, table, reps=20, nc=None, chain=1):
    """Time repeated on-device executions (inputs device-resident, no
    donation, no host transfers in the timed region). `chain` repeats the
    kernel body on-device inside one bass program; time two chain values
    and divide the difference to cancel dispatch overhead. Returns wall
    seconds (min over reps) including the axon dispatch round trip."""
    import time

    import jax
    from jax.sharding import Mesh, NamedSharding, PartitionSpec
    from jax.experimental.shard_map import shard_map

    from concourse import bass2jax
    from concourse.bass2jax import (
        _bass_exec_p,
        install_neuronx_cc_hook,
        partition_id_tensor,
    )

    if nc is None:
        nc = _get_nc(reps=chain)
    install_neuronx_cc_hook()
    in_maps, _, _, _ = _prep(input_batch, table)

    partition_name = (
        nc.partition_id_tensor.name if nc.partition_id_tensor else None
    )
    in_names, out_names, out_avals, zero_outs = [], [], [], []
    for alloc in nc.m.functions[0].allocations:
        if not isinstance(alloc, mybir.MemoryLocationSet):
            continue
        name = alloc.memorylocations[0].name
        if alloc.kind == "ExternalInput":
            if name != partition_name:
                in_names.append(name)
        elif alloc.kind == "ExternalOutput":
            out_names.append(name)
            shape = tuple(alloc.tensor_shape)
            dtype = mybir.dt.np(alloc.dtype)
            out_avals.append(jax.core.ShapedArray(shape, dtype))
            zero_outs.append(np.zeros(shape, dtype))
    n_params = len(in_names)
    all_in_names = in_names + out_names
    if partition_name is not None:
        all_in_names = all_in_names + [partition_name]

    def _body(*args):
        ins_only = list(args[:n_params])
        outs = list(args[n_params:])
        pid = [partition_id_tensor()] if partition_name is not None else []
        operands = ins_only + outs + pid
        outs = list(
            _bass_exec_p.bind(
                *operands,
                out_avals=tuple(out_avals),
                in_names=tuple(all_in_names),
                out_names=tuple(out_names),
                lowering_input_output_aliases=(),
                sim_require_finite=True,
                sim_require_nnan=True,
                nc=nc,
            )
        )
        return tuple(outs)

    devices = jax.devices()[:NCORES]
    mesh = Mesh(np.asarray(devices), ("core",))
    nshard = NamedSharding(mesh, PartitionSpec("core"))
    sharded = jax.jit(
        shard_map(
            _body,
            mesh=mesh,
            in_specs=(PartitionSpec("core"),) * (n_params + len(out_names)),
            out_specs=(PartitionSpec("core"),) * len(out_names),
            check_rep=False,
        ),
        keep_unused=True,
    )
    concat_in = [
        np.concatenate([np.asarray(in_maps[c][nm]) for c in range(NCORES)], axis=0)
        for nm in in_names
    ]
    concat_zeros = [
        np.zeros((NCORES * z.shape[0], *z.shape[1:]), z.dtype) for z in zero_outs
    ]
    dev_args = [jax.device_put(a, nshard) for a in concat_in + concat_zeros]
    jax.block_until_ready(dev_args)
    # warmup (compiles NEFF on first call)
    out = sharded(*dev_args)
    jax.block_until_ready(out)
    times = []
    for _ in range(reps):
        t0 = time.perf_counter()
        out = sharded(*dev_args)
        jax.block_until_ready(out)
        times.append(time.perf_counter() - t0)
    return min(times), times, out


# revision 8
# speedup vs baseline: 5.3131x; 1.1634x over previous
"""Embedding lookup (nn_LookupNetwork) on 8 Trainium2 NeuronCores.

Strategy: vocab-sharded gather. The 100000x128 f32 table is row-sharded
across the 8 cores (12500 rows each), so per-core row indices fit the
int16 index format of the batched SWDGE gather instruction
(InstDMAGatherAnt), which gathers one 512 B table row per index with a
single descriptor instead of one indirect DMA instruction per SBUF
column. The host routes each of the 819200 lookups to the core owning
its table row (sentinel -1 lookups are routed nowhere and stay zero),
ships per-core local indices in the gather's wrapped-by-16 layout, and
scatters the per-core gathered rows back to their original positions.
On-device each core is a pure copy engine: 92 tiles x 1024 indices
(the HW ucode caps one gather at 1024 indices), dma_gather HBM->SBUF
rotated across the 4 SWDGE queues (one queue's 128-slot descriptor ring
otherwise serializes gen against drain), then an HWDGE store SBUF->HBM,
six-way buffered. Measured ~246 us/exec vs the 84.8 ms indirect-DMA
baseline.
"""

import sys

sys.path.insert(0, "/opt/trn_rl_repo")

from contextlib import ExitStack

import numpy as np

import concourse.bacc as bacc
import concourse.bass as bass
import concourse.mybir as mybir
import concourse.tile as tile
from concourse.bass_utils import run_bass_kernel_spmd

VOCAB, D = 100000, 128
BATCH, HIST = 4096, 200
NCORES = 8
P = 128
SHARD = VOCAB // NCORES  # table rows per core
TN = 1024  # lookups per gather tile (HW ucode rejects >1024 idx per gather)
NI = 92 * TN  # padded lookup capacity per core (max shard load ~92.5k)

_nc_cache = {}


def build_nc(bufs=6, reps=1):
    """reps > 1 repeats the whole body on-device (for (t_R - t_1)/(R-1)
    exec timing — the bass2jax hook only allows one bass_exec per jit)."""
    nc = bacc.Bacc(
        "TRN2",
        target_bir_lowering=False,
        debug=False,
        enable_asserts=False,
        num_swdge_queues=4,
    )
    idx_d = nc.dram_tensor(
        "idx", [P, NI // 16], mybir.dt.int16, kind="ExternalInput"
    ).ap()
    tab_d = nc.dram_tensor(
        "tab", [SHARD, D], mybir.dt.float32, kind="ExternalInput"
    ).ap()
    out_d = nc.dram_tensor(
        "out", [NI, D], mybir.dt.float32, kind="ExternalOutput"
    ).ap()

    with tile.TileContext(nc) as tc:
        with ExitStack() as ctx:
            ipool = ctx.enter_context(tc.tile_pool(name="ipool", bufs=2))
            gpool = ctx.enter_context(tc.tile_pool(name="gpool", bufs=bufs))

            for _ in range(reps):
                idx_t = ipool.tile([P, NI // 16], mybir.dt.int16)
                nc.sync.dma_start(idx_t[:], idx_d)

                for t in range(NI // TN):
                    g = gpool.tile([P, (TN // P) * D], mybir.dt.float32)
                    g3 = g[:].rearrange("p (c d) -> p c d", d=D)
                    # Gathered slot i lands at [i % 128, i // 128, :].
                    nc.gpsimd.dma_gather(
                        out_ap=g3,
                        in_ap=tab_d,
                        idxs_ap=idx_t[:, t * (TN // 16) : (t + 1) * (TN // 16)],
                        num_idxs=TN,
                        num_idxs_reg=TN,
                        elem_size=D,
                        queue_num=t % 4,
                    )
                    dst = out_d[t * TN : (t + 1) * TN, :].rearrange(
                        "(c p) d -> p c d", p=P
                    )
                    nc.sync.dma_start(dst, g3)
    nc.compile()
    return nc


def _get_nc(reps=1):
    if reps not in _nc_cache:
        _nc_cache[reps] = build_nc(reps=reps)
    return _nc_cache[reps]


def _prep(input_batch, table):
    """Route lookups to vocab-shard owners; build per-core device inputs."""
    idx = np.asarray(input_batch).reshape(-1).astype(np.int64)
    tab = np.ascontiguousarray(np.asarray(table, dtype=np.float32))
    in_maps, places = [], []
    for c in range(NCORES):
        lo = c * SHARD
        pos = np.nonzero((idx >= lo) & (idx < lo + SHARD))[0]
        over = None
        if len(pos) > NI:  # capacity overflow: excess handled on host
            over = pos[NI:]
            pos = pos[:NI]
        local = (idx[pos] - lo).astype(np.int16)
        buf = np.zeros(NI, np.int16)
        buf[: len(local)] = local
        wrapped = np.ascontiguousarray(
            np.tile(buf.reshape(NI // 16, 16).T, (NCORES, 1))
        )
        in_maps.append({"idx": wrapped, "tab": tab[lo : lo + SHARD]})
        places.append((pos, over))
    return in_maps, places, idx, tab


def kernel(input_batch, table):
    nc = _get_nc()
    in_maps, places, idx, tab = _prep(input_batch, table)
    res = run_bass_kernel_spmd(nc, in_maps, list(range(NCORES)))
    out = np.zeros((BATCH * HIST, D), np.float32)
    for c in range(NCORES):
        pos, over = places[c]
        out[pos] = res.results[c]["out"][: len(pos)]
        if over is not None:
            out[over] = tab[idx[over]]
    return out.reshape(BATCH, HIST, D)


def bench(input_batch, table, reps=20, nc=None, chain=1):
    """Time repeated on-device executions (inputs device-resident, no
    donation, no host transfers in the timed region). `chain` repeats the
    kernel body on-device inside one bass program; time two chain values
    and divide the difference to cancel dispatch overhead. Returns wall
    seconds (min over reps) including the axon dispatch round trip."""
    import time

    import jax
    from jax.sharding import Mesh, NamedSharding, PartitionSpec
    from jax.experimental.shard_map import shard_map

    from concourse import bass2jax
    from concourse.bass2jax import (
        _bass_exec_p,
        install_neuronx_cc_hook,
        partition_id_tensor,
    )

    if nc is None:
        nc = _get_nc(reps=chain)
    install_neuronx_cc_hook()
    in_maps, _, _, _ = _prep(input_batch, table)

    partition_name = (
        nc.partition_id_tensor.name if nc.partition_id_tensor else None
    )
    in_names, out_names, out_avals, zero_outs = [], [], [], []
    for alloc in nc.m.functions[0].allocations:
        if not isinstance(alloc, mybir.MemoryLocationSet):
            continue
        name = alloc.memorylocations[0].name
        if alloc.kind == "ExternalInput":
            if name != partition_name:
                in_names.append(name)
        elif alloc.kind == "ExternalOutput":
            out_names.append(name)
            shape = tuple(alloc.tensor_shape)
            dtype = mybir.dt.np(alloc.dtype)
            out_avals.append(jax.core.ShapedArray(shape, dtype))
            zero_outs.append(np.zeros(shape, dtype))
    n_params = len(in_names)
    all_in_names = in_names + out_names
    if partition_name is not None:
        all_in_names = all_in_names + [partition_name]

    def _body(*args):
        ins_only = list(args[:n_params])
        outs = list(args[n_params:])
        pid = [partition_id_tensor()] if partition_name is not None else []
        operands = ins_only + outs + pid
        outs = list(
            _bass_exec_p.bind(
                *operands,
                out_avals=tuple(out_avals),
                in_names=tuple(all_in_names),
                out_names=tuple(out_names),
                lowering_input_output_aliases=(),
                sim_require_finite=True,
                sim_require_nnan=True,
                nc=nc,
            )
        )
        return tuple(outs)

    devices = jax.devices()[:NCORES]
    mesh = Mesh(np.asarray(devices), ("core",))
    nshard = NamedSharding(mesh, PartitionSpec("core"))
    sharded = jax.jit(
        shard_map(
            _body,
            mesh=mesh,
            in_specs=(PartitionSpec("core"),) * (n_params + len(out_names)),
            out_specs=(PartitionSpec("core"),) * len(out_names),
            check_rep=False,
        ),
        keep_unused=True,
    )
    concat_in = [
        np.concatenate([np.asarray(in_maps[c][nm]) for c in range(NCORES)], axis=0)
        for nm in in_names
    ]
    concat_zeros = [
        np.zeros((NCORES * z.shape[0], *z.shape[1:]), z.dtype) for z in zero_outs
    ]
    dev_args = [jax.device_put(a, nshard) for a in concat_in + concat_zeros]
    jax.block_until_ready(dev_args)
    # warmup (compiles NEFF on first call)
    out = sharded(*dev_args)
    jax.block_until_ready(out)
    times = []
    for _ in range(reps):
        t0 = time.perf_counter()
        out = sharded(*dev_args)
        jax.block_until_ready(out)
        times.append(time.perf_counter() - t0)
    return min(times), times, out


# revision 10
# speedup vs baseline: 13.7472x; 2.5874x over previous
"""Embedding lookup (nn_LookupNetwork) on 8 Trainium2 NeuronCores.

Strategy: vocab-sharded gather. The 100000x128 f32 table is row-sharded
across the 8 cores (12500 rows each), so per-core row indices fit the
int16 index format of the batched SWDGE gather instruction
(InstDMAGatherAnt), which gathers one 512 B table row per index with a
single descriptor instead of one indirect DMA instruction per SBUF
column. The host routes each of the 819200 lookups to the core owning
its table row (sentinel -1 lookups are routed nowhere and stay zero),
ships per-core local indices in the gather's wrapped-by-16 layout, and
scatters the per-core gathered rows back to their original positions.
On-device each core is a pure copy engine: 92 tiles x 1024 indices
(the HW ucode caps one gather at 1024 indices), dma_gather HBM->SBUF
rotated across the 4 SWDGE queues (one queue's 128-slot descriptor ring
otherwise serializes gen against drain), then an HWDGE store SBUF->HBM,
six-way buffered, multi-packet descriptors for smoother SDMA
interleaving. Measured ~212 us/exec vs the 84.8 ms indirect-DMA
baseline.
"""

import sys

sys.path.insert(0, "/opt/trn_rl_repo")

from contextlib import ExitStack

import numpy as np

import concourse.bacc as bacc
import concourse.bass as bass
import concourse.mybir as mybir
import concourse.tile as tile
from concourse.bass_utils import run_bass_kernel_spmd

VOCAB, D = 100000, 128
BATCH, HIST = 4096, 200
NCORES = 8
P = 128
SHARD = VOCAB // NCORES  # table rows per core
TN = 1024  # lookups per gather tile (HW ucode rejects >1024 idx per gather)
NI = 92 * TN  # padded lookup capacity per core (max shard load ~92.5k)

_nc_cache = {}


def build_nc(bufs=6, reps=1):
    """reps > 1 repeats the whole body on-device (for (t_R - t_1)/(R-1)
    exec timing — the bass2jax hook only allows one bass_exec per jit)."""
    nc = bacc.Bacc(
        "TRN2",
        target_bir_lowering=False,
        debug=False,
        enable_asserts=False,
        num_swdge_queues=4,
    )
    idx_d = nc.dram_tensor(
        "idx", [P, NI // 16], mybir.dt.int16, kind="ExternalInput"
    ).ap()
    tab_d = nc.dram_tensor(
        "tab", [SHARD, D], mybir.dt.float32, kind="ExternalInput"
    ).ap()
    out_d = nc.dram_tensor(
        "out", [NI, D], mybir.dt.float32, kind="ExternalOutput"
    ).ap()

    with tile.TileContext(nc) as tc:
        with ExitStack() as ctx:
            ipool = ctx.enter_context(tc.tile_pool(name="ipool", bufs=2))
            gpool = ctx.enter_context(tc.tile_pool(name="gpool", bufs=bufs))

            for _ in range(reps):
                idx_t = ipool.tile([P, NI // 16], mybir.dt.int16)
                nc.sync.dma_start(idx_t[:], idx_d)

                for t in range(NI // TN):
                    g = gpool.tile([P, (TN // P) * D], mybir.dt.float32)
                    g3 = g[:].rearrange("p (c d) -> p c d", d=D)
                    # Gathered slot i lands at [i % 128, i // 128, :].
                    nc.gpsimd.dma_gather(
                        out_ap=g3,
                        in_ap=tab_d,
                        idxs_ap=idx_t[:, t * (TN // 16) : (t + 1) * (TN // 16)],
                        num_idxs=TN,
                        num_idxs_reg=TN,
                        elem_size=D,
                        queue_num=t % 4,
                        single_packet=False,
                    )
                    dst = out_d[t * TN : (t + 1) * TN, :].rearrange(
                        "(c p) d -> p c d", p=P
                    )
                    nc.sync.dma_start(dst, g3)
    nc.compile()
    return nc


def _get_nc(reps=1):
    if reps not in _nc_cache:
        _nc_cache[reps] = build_nc(reps=reps)
    return _nc_cache[reps]


def _prep(input_batch, table):
    """Route lookups to vocab-shard owners; build per-core device inputs."""
    idx = np.asarray(input_batch).reshape(-1).astype(np.int64)
    tab = np.ascontiguousarray(np.asarray(table, dtype=np.float32))
    in_maps, places = [], []
    for c in range(NCORES):
        lo = c * SHARD
        pos = np.nonzero((idx >= lo) & (idx < lo + SHARD))[0]
        over = None
        if len(pos) > NI:  # capacity overflow: excess handled on host
            over = pos[NI:]
            pos = pos[:NI]
        local = (idx[pos] - lo).astype(np.int16)
        buf = np.zeros(NI, np.int16)
        buf[: len(local)] = local
        wrapped = np.ascontiguousarray(
            np.tile(buf.reshape(NI // 16, 16).T, (NCORES, 1))
        )
        in_maps.append({"idx": wrapped, "tab": tab[lo : lo + SHARD]})
        places.append((pos, over))
    return in_maps, places, idx, tab


def kernel(input_batch, table):
    nc = _get_nc()
    in_maps, places, idx, tab = _prep(input_batch, table)
    res = run_bass_kernel_spmd(nc, in_maps, list(range(NCORES)))
    out = np.zeros((BATCH * HIST, D), np.float32)
    for c in range(NCORES):
        pos, over = places[c]
        out[pos] = res.results[c]["out"][: len(pos)]
        if over is not None:
            out[over] = tab[idx[over]]
    return out.reshape(BATCH, HIST, D)


def bench(input_batch, table, reps=20, nc=None, chain=1):
    """Time repeated on-device executions (inputs device-resident, no
    donation, no host transfers in the timed region). `chain` repeats the
    kernel body on-device inside one bass program; time two chain values
    and divide the difference to cancel dispatch overhead. Returns wall
    seconds (min over reps) including the axon dispatch round trip."""
    import time

    import jax
    from jax.sharding import Mesh, NamedSharding, PartitionSpec
    from jax.experimental.shard_map import shard_map

    from concourse import bass2jax
    from concourse.bass2jax import (
        _bass_exec_p,
        install_neuronx_cc_hook,
        partition_id_tensor,
    )

    if nc is None:
        nc = _get_nc(reps=chain)
    install_neuronx_cc_hook()
    in_maps, _, _, _ = _prep(input_batch, table)

    partition_name = (
        nc.partition_id_tensor.name if nc.partition_id_tensor else None
    )
    in_names, out_names, out_avals, zero_outs = [], [], [], []
    for alloc in nc.m.functions[0].allocations:
        if not isinstance(alloc, mybir.MemoryLocationSet):
            continue
        name = alloc.memorylocations[0].name
        if alloc.kind == "ExternalInput":
            if name != partition_name:
                in_names.append(name)
        elif alloc.kind == "ExternalOutput":
            out_names.append(name)
            shape = tuple(alloc.tensor_shape)
            dtype = mybir.dt.np(alloc.dtype)
            out_avals.append(jax.core.ShapedArray(shape, dtype))
            zero_outs.append(np.zeros(shape, dtype))
    n_params = len(in_names)
    all_in_names = in_names + out_names
    if partition_name is not None:
        all_in_names = all_in_names + [partition_name]

    def _body(*args):
        ins_only = list(args[:n_params])
        outs = list(args[n_params:])
        pid = [partition_id_tensor()] if partition_name is not None else []
        operands = ins_only + outs + pid
        outs = list(
            _bass_exec_p.bind(
                *operands,
                out_avals=tuple(out_avals),
                in_names=tuple(all_in_names),
                out_names=tuple(out_names),
                lowering_input_output_aliases=(),
                sim_require_finite=True,
                sim_require_nnan=True,
                nc=nc,
            )
        )
        return tuple(outs)

    devices = jax.devices()[:NCORES]
    mesh = Mesh(np.asarray(devices), ("core",))
    nshard = NamedSharding(mesh, PartitionSpec("core"))
    sharded = jax.jit(
        shard_map(
            _body,
            mesh=mesh,
            in_specs=(PartitionSpec("core"),) * (n_params + len(out_names)),
            out_specs=(PartitionSpec("core"),) * len(out_names),
            check_rep=False,
        ),
        keep_unused=True,
    )
    concat_in = [
        np.concatenate([np.asarray(in_maps[c][nm]) for c in range(NCORES)], axis=0)
        for nm in in_names
    ]
    concat_zeros = [
        np.zeros((NCORES * z.shape[0], *z.shape[1:]), z.dtype) for z in zero_outs
    ]
    dev_args = [jax.device_put(a, nshard) for a in concat_in + concat_zeros]
    jax.block_until_ready(dev_args)
    # warmup (compiles NEFF on first call)
    out = sharded(*dev_args)
    jax.block_until_ready(out)
    times = []
    for _ in range(reps):
        t0 = time.perf_counter()
        out = sharded(*dev_args)
        jax.block_until_ready(out)
        times.append(time.perf_counter() - t0)
    return min(times), times, out
